# revision 46
# baseline (speedup 1.0000x reference)
"""MoE (top-2 of 16 routed experts + 2 shared experts) Trainium2 kernel.

Strategy: expert-parallel over 8 NeuronCores, token dispatch on host (the
router is 0.01% of the FLOPs; host-side routing lets each core receive
exactly the tokens it needs, already gathered, transposed and packed).

Per core (SPMD program, identical on all cores; per-core in_maps differ):
  slot "ra": routed expert (one of the 8 least-loaded)   -- fp8 DoubleRow
  slot "rb": routed expert (one of the 8 most-loaded)    -- fp8 DoubleRow
  slot "sh": one shared expert on one quarter of tokens  -- bf16
Each slot is a dense FFN in feature-major ("transposed") layout:
  mid^T[d,t] = gelu( sum_h Wup[h,d] * x^T[h,t] )
  y^T[h,t]   =       sum_d Wdn[d,h] * mid^T[d,t]
Host scatter-adds y^T into the [T,H] output with the router gate weights
(gelu is the only nonlinearity, so the per-token gate commutes with the
down projection).

The routed path runs in fp8-e4m3 with MatmulPerfMode.DoubleRow: each
matmul contracts 256 rows (two 128-blocks packed per PE cell), doubling
tensor-engine throughput.  Inputs are pre-scaled on host (x*16, W*512,
powers of two) to center values in e4m3's range; the up-psum is descaled
inside the fused gelu (scale=1/8192) and the 512x down-scale is folded
into the host-side gate multiply.  The routed output is ~19% of the total
norm, so its ~5% fp8 error contributes ~1.1e-2 overall -- under the 2e-2
budget -- while the shared path stays bf16 (~3.4e-3).

Up-projection weights are packed dj-slab-major ([128, DT, HT, 128]) so
the first matmul group only needs slab 0 (256KB) + half an x chunk, not
the full weight tensor: compute starts ~10us in instead of ~30us.
"""

import numpy as np
import ml_dtypes

import concourse.mybir as mybir
import concourse.tile as tile
from concourse import bacc
from concourse.bass_utils import run_bass_kernel_spmd

BF16 = mybir.dt.bfloat16
FP8 = mybir.dt.float8e4
F32 = mybir.dt.float32
NP_BF16 = ml_dtypes.bfloat16
NP_FP8 = ml_dtypes.float8_e4m3
DR = mybir.MatmulPerfMode.DoubleRow
GELU = mybir.ActivationFunctionType.Gelu

B, S, H, D = 4, 1024, 2048, 1024
T = B * S
E_RT, E_SH, CORES = 16, 2, 8
HT, DT = H // 128, D // 128  # h-tiles (16), d-tiles (8)
N_SH = T // (CORES // E_SH)  # shared-slot tokens per core (1024)
TT = 512                     # max moving-operand / psum tile width
YG = 4                       # output h-tiles staged per store DMA
WARM = 16                    # PE warmup matmuls (cover DMA head + HAM ramp)

XS, WS = 16.0, 512.0         # fp8 pre-scales (powers of two)
YS = 16.0                    # routed y store scale: y_fp8 = psum / YS

_prog_cache = {}
LAST_RESULTS = None  # BassKernelResults of the most recent run (for test.py)


def _chunks(n, tt=TT):
    """Split n (a multiple of 16) into the fewest chunks of width <= tt,
    all widths multiples of 16 (the fp8 DoubleRow pair-stride must be
    16B-aligned)."""
    k = -(-n // tt)
    w = -(-(-(-n // k)) // 16) * 16
    out, off = [], 0
    while off < n:
        cw = min(w, n - off)
        out.append((off, cw))
        off += cw
    return out


# ra/sh x chunks are 256 wide: smaller first-matmul gating loads, and the
# x pool's slot size stays 8KB/partition (bf16 256 == fp8 512 bytes);
# per-column matmul throughput is width-independent.  rb keeps 512 (576
# splits into 2x288).
SLOT_TT = {"ra": TT, "sh": 256, "rb": TT}


def _build_program(cap_a, cap_b):
    nc = bacc.Bacc("TRN2", target_bir_lowering=False, debug=False,
                   num_devices=CORES)
    slots = []
    for name, n, fp8 in (("ra", cap_a, True), ("sh", N_SH, False),
                         ("rb", cap_b, True)):
        dt = FP8 if fp8 else BF16
        xd = nc.dram_tensor(f"x_{name}", [128, HT * n], dt,
                            kind="ExternalInput")
        wu = nc.dram_tensor(f"wup_{name}", [128, DT * HT * 128], dt,
                            kind="ExternalInput")
        wd = nc.dram_tensor(f"wdn_{name}", [128, DT * H], dt,
                            kind="ExternalInput")
        yd = nc.dram_tensor(f"y_{name}", [128, HT * n], dt,
                            kind="ExternalOutput")
        slots.append((name, n, fp8, xd, wu, wd, yd))

    with tile.TileContext(nc) as tc:
        with (
            tc.tile_pool(name="wpool_r", bufs=2) as wpool_r,
            tc.tile_pool(name="wpool_sh", bufs=1) as wpool_sh,
            tc.tile_pool(name="xpool", bufs=4) as xpool,
            tc.tile_pool(name="mpool", bufs=5) as mpool,
            tc.tile_pool(name="ypool", bufs=4) as ypool,
            tc.tile_pool(name="ps1pool", bufs=3, space="PSUM") as ps1pool,
            tc.tile_pool(name="ps2pool", bufs=3, space="PSUM") as ps2pool,
        ):
            # DMA orchestration.  Three HWDGE rings (SP, ACT, GPSIMD DGE);
            # each processes its DMAs in issue order, and each ramps at
            # only ~70-150GB/s for the first ~20us (aggregate early HBM
            # ~250-300GB/s/core), so loads are laid out across all three
            # rings in strict global deadline order:
            #   ACT:    ra x, rb slabs+x (issues finish before the first
            #           gelu queues on this engine), then all y stores
            #   sync:   ra slabs, then the sh bulk (x1, slabs, x2)
            #   GPSIMD: all down-weights (deadlines pushed past 55us by
            #           the phase interleave) + late sh x chunks
            # Keep >=4KB per partition-descriptor — smaller descriptors
            # are descriptor-rate-bound (~50GB/s/queue vs 300+ for 8KB+).
            def emit_loads(si):
                name, n, fp8, xd, wu, wd, yd = slots[si]
                dt = FP8 if fp8 else BF16
                wpool = wpool_r if fp8 else wpool_sh
                tt = SLOT_TT[name]
                wut = wpool.tile([128, DT, HT, 128], dt, tag="wup",
                                 name=f"wup_{name}")
                wdt = wpool.tile([128, DT, H], dt, tag="wdn",
                                 name=f"wdn_{name}")
                xts = []

                def load_slab(dj, halves=False):
                    sw = HT * 128
                    hh = HT // 2
                    pieces = ([(0, hh), (hh, HT - hh)] if halves
                              else [(0, HT)])
                    for h0, hn in pieces:
                        nc.sync.dma_start(
                            out=wut[:, dj, h0:h0 + hn, :],
                            in_=wu[:, dj * sw + h0 * 128:
                                   dj * sw + (h0 + hn) * 128].rearrange(
                                "p (h c) -> p h c", h=hn))

                def load_x(ci, eng, parts=2):
                    off, w = _chunks(n, tt)[ci]
                    xt = xpool.tile([128, HT, w], dt, tag="x",
                                    name=f"x_{name}_{off}")
                    xts.append(xt)
                    hh = HT // parts
                    for pi in range(parts):
                        h0 = pi * hh
                        eng.dma_start(
                            out=xt[:, h0:h0 + hh, :],
                            in_=xd[:, HT * off + h0 * w:
                                   HT * off + (h0 + hh) * w].rearrange(
                                "p (h w) -> p h w", h=hh))

                def load_wdn(gi, eng):
                    dg = DT // 2
                    g = gi * dg
                    eng.dma_start(
                        out=wdt[:, g:g + dg, :],
                        in_=wd[:, g * H:(g + dg) * H].rearrange(
                            "p (c h) -> p c h", c=dg))

                nchunks = len(_chunks(n, tt))
                if si == 0:
                    # first slot: slabs on sync, x on the GPSIMD ring —
                    # both rings ramp in parallel and the first matmul
                    # group only waits for slab0-half + one x quarter.
                    # Down-weights ride the otherwise-idle ACT ring: both
                    # issues complete by ~8us (before the first gelu
                    # queues there) and the transfers land by ~22us,
                    # beating the ~27us down-phase deadline that the
                    # GPSIMD ring (busy with x) kept missing by 4-8us.
                    load_slab(0, halves=True)
                    load_x(0, nc.gpsimd, parts=4)
                    for dj in range(1, DT):
                        load_slab(dj)
                    for ci in range(1, nchunks):
                        load_x(ci, nc.gpsimd)
                    load_wdn(0, nc.scalar)
                    load_wdn(1, nc.scalar)
                elif name == "sh":
                    # bulk bf16 slot, both rings in deadline order.  sync:
                    # x1 before the slabs (all needed at slot start).
                    # GPSIMD: late x chunks BEFORE the down-weights — their
                    # pool-slot waits also throttle that ring through the
                    # early HBM crunch, and the ups-then-downs chunk order
                    # pushes the down-weight deadline to slot-start+56us.
                    load_x(0, nc.sync)
                    for dj in range(DT):
                        load_slab(dj)
                    if nchunks > 1:
                        load_x(1, nc.sync)
                    for ci in range(2, nchunks):
                        load_x(ci, nc.gpsimd)
                    load_wdn(0, nc.gpsimd)
                    load_wdn(1, nc.gpsimd)
                else:
                    for dj in range(DT):
                        load_slab(dj)
                    for ci in range(nchunks):
                        load_x(ci, nc.sync)
                    load_wdn(0, nc.gpsimd)
                    load_wdn(1, nc.gpsimd)
                return wut, wdt, xts

            # Dummy matmuls on scratch tiles: the PE HAM clock-gate only
            # lifts to 2.4 GHz after ~3.4us of sustained activity, so warm
            # it up while the first loads stream in.
            wlhs = xpool.tile([128, 128], BF16, tag="warm_l", bufs=1,
                              name="warm_lhs")
            wrhs = xpool.tile([128, TT], BF16, tag="warm_r", bufs=1,
                              name="warm_rhs")
            nc.vector.memset(wlhs[:], 0)
            nc.vector.memset(wrhs[:], 0)
            # preload the ACT gelu table (1.3us lazy load otherwise stalls
            # the first real activation)
            wgel = xpool.tile([128, 16], BF16, tag="warm_g", bufs=1,
                              name="warm_gelu")
            nc.scalar.activation(wgel[:], wrhs[:, :16], GELU)
            wps = ps1pool.tile([128, TT], F32, tag="ps1", name="warm_ps")
            for wi in range(WARM):
                nc.tensor.matmul(wps[:], lhsT=wlhs[:], rhs=wrhs[:],
                                 start=True, stop=True)

            loaded = [emit_loads(si) for si in range(len(slots))]

            def up_phase(slot, wut, xt, ci):
                name, n, fp8, xd, wu, wd, yd = slot
                off, w = _chunks(n, SLOT_TT[name])[ci]
                mid = mpool.tile([128, DT, w], FP8 if fp8 else BF16,
                                 tag="mid", name=f"mid_{name}_{off}")
                for dj in range(DT):
                    ps = ps1pool.tile([128, TT], F32, tag="ps1",
                                      name=f"ps1_{name}_{off}_{dj}")
                    if fp8:
                        for hq in range(HT // 2):
                            nc.tensor.matmul(
                                ps[:, :w],
                                lhsT=wut[:, dj, 2 * hq:2 * hq + 2, :],
                                rhs=xt[:, 2 * hq:2 * hq + 2, :],
                                start=(hq == 0),
                                stop=(hq == HT // 2 - 1),
                                perf_mode=DR,
                            )
                    else:
                        for hi in range(HT):
                            nc.tensor.matmul(
                                ps[:, :w],
                                lhsT=wut[:, dj, hi, :],
                                rhs=xt[:, hi, :],
                                start=(hi == 0),
                                stop=(hi == HT - 1),
                            )
                    nc.scalar.activation(
                        mid[:, dj, :], ps[:, :w], GELU,
                        scale=float(1 / (XS * WS)) if fp8 else 1.0)
                return mid

            def down_phase(slot, wdt, mid, ci, yg):
                name, n, fp8, xd, wu, wd, yd = slot
                off, w = _chunks(n, SLOT_TT[name])[ci]
                base = HT * off
                for hi in range(HT):
                    ps2 = ps2pool.tile([128, TT], F32, tag="ps2",
                                       name=f"ps2_{name}_{off}_{hi}")
                    if fp8:
                        for t in range(DT // 2):
                            nc.tensor.matmul(
                                ps2[:, :w],
                                lhsT=wdt[:, 2 * t:2 * t + 2,
                                         hi * 128:(hi + 1) * 128],
                                rhs=mid[:, 2 * t:2 * t + 2, :],
                                start=(t == 0),
                                stop=(t == DT // 2 - 1),
                                perf_mode=DR,
                            )
                    else:
                        for dj in range(DT):
                            nc.tensor.matmul(
                                ps2[:, :w],
                                lhsT=wdt[:, dj, hi * 128:(hi + 1) * 128],
                                rhs=mid[:, dj, :],
                                start=(dj == 0),
                                stop=(dj == DT - 1),
                            )
                    g = hi % yg
                    if g == 0:
                        yt = ypool.tile([128, yg, w],
                                        FP8 if fp8 else BF16, tag="y",
                                        name=f"y_{name}_{off}_{hi}")
                    if fp8:
                        nc.vector.tensor_scalar_mul(
                            yt[:, g, :], ps2[:, :w], float(1 / YS))
                    else:
                        nc.vector.tensor_copy(yt[:, g, :], ps2[:, :w])
                    if g == yg - 1:
                        # store on the ACT HWDGE ring (sync ring carries
                        # the loads)
                        lo = base + (hi - g) * w
                        nc.scalar.dma_start(
                            out=yd[:, lo:lo + yg * w].rearrange(
                                "p (h w) -> p h w", h=yg),
                            in_=yt[:])

            for si, slot in enumerate(slots):
                name, n, fp8, xd, wu, wd, yd = slot
                wut, wdt, xts = loaded[si]
                last_slot = si == len(slots) - 1
                nchunks = len(_chunks(n, SLOT_TT[name]))
                if name == "sh":
                    # all ups, then all downs: pushes the 4MB down-weight
                    # deadline from slot-start+14us to +56us, easing the
                    # early HBM crunch
                    mids = [up_phase(slot, wut, xts[ci], ci)
                            for ci in range(nchunks)]
                    for ci in range(nchunks):
                        down_phase(slot, wdt, mids[ci], ci, YG)
                else:
                    for ci in range(nchunks):
                        yg = 2 if (last_slot and ci == nchunks - 1) else YG
                        mid = up_phase(slot, wut, xts[ci], ci)
                        down_phase(slot, wdt, mid, ci, yg)
    nc.compile()
    return nc


def _pack_rows(a, nt):
    """[nt*128, m] row-major -> [128, nt*m] with per-partition contiguous
    (tile-major) layout."""
    m = a.shape[1]
    return np.ascontiguousarray(
        a.reshape(nt, 128, m).transpose(1, 0, 2).reshape(128, nt * m))


def _pack_slab(wu):
    """Up-weights [H, D] -> [128, DT*HT*128], dj-slab-major: element
    (k, dj, ht, c) = wu[ht*128+k, dj*128+c]."""
    a = wu.reshape(HT, 128, DT, 128).transpose(1, 2, 0, 3)
    return np.ascontiguousarray(a.reshape(128, DT * HT * 128))


def _pack_x(xTc, tt=TT):
    """[H, n] -> [128, HT*n] chunk-major."""
    n = xTc.shape[1]
    parts = [_pack_rows(xTc[:, off:off + w], HT)
             for off, w in _chunks(n, tt)]
    return np.ascontiguousarray(np.concatenate(parts, axis=1))


def _unpack_y(yflat, n, tt=TT):
    """[128, HT*n] chunk-major -> [n, H] (token-major)."""
    yflat = yflat.astype(np.float32)
    out = np.empty((n, H), np.float32)
    base = 0
    for off, w in _chunks(n, tt):
        blk = yflat[:, base:base + HT * w].reshape(128, HT, w)
        out[off:off + w] = blk.transpose(2, 1, 0).reshape(w, H)
        base += HT * w
    return out


def _to_fp8(a, scale):
    return np.clip(a * np.float32(scale), -240, 240).astype(NP_FP8)


def _route(x2d, w_router):
    """Top-2 routing, matching the reference's softmax-then-top_k."""
    logits = x2d @ w_router
    m = logits.max(-1, keepdims=True)
    e = np.exp(logits - m)
    probs = e / e.sum(-1, keepdims=True)
    rows = np.arange(x2d.shape[0])
    i1 = probs.argmax(-1)
    masked = probs.copy()
    masked[rows, i1] = -np.inf
    i2 = masked.argmax(-1)
    return probs, i1, i2


def kernel(x, Wsh_up, Wsh_down, Wrt_up, Wrt_down, W_router):
    global LAST_RESULTS
    x = np.asarray(x, np.float32)
    Wsh_up = np.asarray(Wsh_up, np.float32)
    Wsh_down = np.asarray(Wsh_down, np.float32)
    Wrt_up = np.asarray(Wrt_up, np.float32)
    Wrt_down = np.asarray(Wrt_down, np.float32)
    W_router = np.asarray(W_router, np.float32)

    x2d = x.reshape(T, H)
    probs, i1, i2 = _route(x2d, W_router)

    # token ids / gate values per routed expert
    ids, gates = [], []
    for e in range(E_RT):
        sel = np.where((i1 == e) | (i2 == e))[0]
        ids.append(sel)
        gates.append(probs[sel, e].astype(np.float32))

    # slot "rb" takes the 8 most-loaded experts, "ra" the 8 least-loaded,
    # so the two static capacities hug the actual counts ("ra" usually
    # becomes a single full-width 512 chunk and runs first).
    order = sorted(range(E_RT), key=lambda e: -len(ids[e]))
    slot_experts = {"rb": order[:CORES], "ra": order[CORES:]}
    caps = {}
    for slot in ("ra", "rb"):
        mx = max(len(ids[e]) for e in slot_experts[slot])
        caps[slot] = max(512, -(-mx // 32) * 32)

    key = (caps["ra"], caps["rb"])
    if key not in _prog_cache:
        _prog_cache[key] = _build_program(*key)
    nc = _prog_cache[key]

    xbf = x2d.astype(NP_BF16)

    in_maps = []
    for c in range(CORES):
        se, q = c % E_SH, c // E_SH
        m = {
            "x_sh": _pack_x(np.ascontiguousarray(
                xbf[q * N_SH:(q + 1) * N_SH].T), SLOT_TT["sh"]),
            "wup_sh": _pack_slab(Wsh_up[se]).astype(NP_BF16),
            "wdn_sh": _pack_rows(Wsh_down[se], DT).astype(NP_BF16),
        }
        for slot in ("ra", "rb"):
            e = slot_experts[slot][c]
            sel = ids[e]
            cap = caps[slot]
            xe = np.zeros((H, cap), NP_FP8)
            xe[:, :len(sel)] = _to_fp8(x2d[sel].T, XS)
            m[f"x_{slot}"] = _pack_x(xe, SLOT_TT[slot])
            m[f"wup_{slot}"] = _to_fp8(_pack_slab(Wrt_up[e]), WS)
            m[f"wdn_{slot}"] = _to_fp8(_pack_rows(Wrt_down[e], DT), WS)
        in_maps.append(m)

    res = run_bass_kernel_spmd(nc, in_maps, core_ids=list(range(CORES)))
    LAST_RESULTS = res

    out = np.zeros((T, H), np.float32)
    for c in range(CORES):
        q = c // E_SH
        out[q * N_SH:(q + 1) * N_SH] += _unpack_y(res.results[c]["y_sh"],
                                                  N_SH, SLOT_TT["sh"])
    for slot in ("ra", "rb"):
        for c in range(CORES):
            e = slot_experts[slot][c]
            sel = ids[e]
            y = _unpack_y(res.results[c][f"y_{slot}"], caps[slot],
                          SLOT_TT[slot])
            out[sel] += (gates[e] * (YS / WS))[:, None] * y[:len(sel)]
    return out.reshape(B, S, H)


# revision 49
# speedup vs baseline: 1.0151x; 1.0151x over previous
"""MoE (top-2 of 16 routed experts + 2 shared experts) Trainium2 kernel.

Strategy: expert-parallel over 8 NeuronCores, token dispatch on host (the
router is 0.01% of the FLOPs; host-side routing lets each core receive
exactly the tokens it needs, already gathered, transposed and packed).

Per core (SPMD program, identical on all cores; per-core in_maps differ):
  slot "ra": routed expert (one of the 8 least-loaded)   -- fp8 DoubleRow
  slot "rb": routed expert (one of the 8 most-loaded)    -- fp8 DoubleRow
  slot "sh": one shared expert on one quarter of tokens  -- bf16
Each slot is a dense FFN in feature-major ("transposed") layout:
  mid^T[d,t] = gelu( sum_h Wup[h,d] * x^T[h,t] )
  y^T[h,t]   =       sum_d Wdn[d,h] * mid^T[d,t]
Host scatter-adds y^T into the [T,H] output with the router gate weights
(gelu is the only nonlinearity, so the per-token gate commutes with the
down projection).

The routed path runs in fp8-e4m3 with MatmulPerfMode.DoubleRow: each
matmul contracts 256 rows (two 128-blocks packed per PE cell), doubling
tensor-engine throughput.  Inputs are pre-scaled on host (x*16, W*512,
powers of two) to center values in e4m3's range; the up-psum is descaled
inside the fused gelu (scale=1/8192) and the 512x down-scale is folded
into the host-side gate multiply.  The routed output is ~19% of the total
norm, so its ~5% fp8 error contributes ~1.1e-2 overall -- under the 2e-2
budget -- while the shared path stays bf16 (~3.4e-3).

Up-projection weights are packed dj-slab-major ([128, DT, HT, 128]) so
the first matmul group only needs slab 0 (256KB) + half an x chunk, not
the full weight tensor: compute starts ~10us in instead of ~30us.
"""

import numpy as np
import ml_dtypes

import concourse.mybir as mybir
import concourse.tile as tile
from concourse import bacc
from concourse.bass_utils import run_bass_kernel_spmd

BF16 = mybir.dt.bfloat16
FP8 = mybir.dt.float8e4
F32 = mybir.dt.float32
NP_BF16 = ml_dtypes.bfloat16
NP_FP8 = ml_dtypes.float8_e4m3
DR = mybir.MatmulPerfMode.DoubleRow
GELU = mybir.ActivationFunctionType.Gelu

B, S, H, D = 4, 1024, 2048, 1024
T = B * S
E_RT, E_SH, CORES = 16, 2, 8
HT, DT = H // 128, D // 128  # h-tiles (16), d-tiles (8)
N_SH = T // (CORES // E_SH)  # shared-slot tokens per core (1024)
TT = 512                     # max moving-operand / psum tile width
YG = 4                       # output h-tiles staged per store DMA
WARM = 16                    # PE warmup matmuls (cover DMA head + HAM ramp)

XS, WS = 16.0, 512.0         # fp8 pre-scales (powers of two)
YS = 16.0                    # routed y store scale: y_fp8 = psum / YS

_prog_cache = {}
LAST_RESULTS = None  # BassKernelResults of the most recent run (for test.py)


def _chunks(n, tt=TT):
    """Split n (a multiple of 16) into the fewest chunks of width <= tt,
    all widths multiples of 16 (the fp8 DoubleRow pair-stride must be
    16B-aligned)."""
    k = -(-n // tt)
    w = -(-(-(-n // k)) // 16) * 16
    out, off = [], 0
    while off < n:
        cw = min(w, n - off)
        out.append((off, cw))
        off += cw
    return out


# ra/sh x chunks are 256 wide: smaller first-matmul gating loads, and the
# x pool's slot size stays 8KB/partition (bf16 256 == fp8 512 bytes);
# per-column matmul throughput is width-independent.  rb keeps 512 (576
# splits into 2x288).
SLOT_TT = {"ra": TT, "sh": 256, "rb": TT}


def _build_program(cap_a, cap_b):
    nc = bacc.Bacc("TRN2", target_bir_lowering=False, debug=False,
                   num_devices=CORES)
    slots = []
    for name, n, fp8 in (("ra", cap_a, True), ("sh", N_SH, False),
                         ("rb", cap_b, True)):
        dt = FP8 if fp8 else BF16
        xd = nc.dram_tensor(f"x_{name}", [128, HT * n], dt,
                            kind="ExternalInput")
        wu = nc.dram_tensor(f"wup_{name}", [128, DT * HT * 128], dt,
                            kind="ExternalInput")
        wd = nc.dram_tensor(f"wdn_{name}", [128, DT * H], dt,
                            kind="ExternalInput")
        yd = nc.dram_tensor(f"y_{name}", [128, HT * n], dt,
                            kind="ExternalOutput")
        slots.append((name, n, fp8, xd, wu, wd, yd))

    with tile.TileContext(nc) as tc:
        with (
            tc.tile_pool(name="wpool_r", bufs=2) as wpool_r,
            tc.tile_pool(name="wpool_sh", bufs=1) as wpool_sh,
            tc.tile_pool(name="xpool", bufs=4) as xpool,
            tc.tile_pool(name="mpool", bufs=5) as mpool,
            tc.tile_pool(name="ypool", bufs=4) as ypool,
            tc.tile_pool(name="ps1pool", bufs=3, space="PSUM") as ps1pool,
            tc.tile_pool(name="ps2pool", bufs=3, space="PSUM") as ps2pool,
        ):
            # DMA orchestration.  Three HWDGE rings (SP, ACT, GPSIMD DGE);
            # each processes its DMAs in issue order, and each ramps at
            # only ~70-150GB/s for the first ~20us (aggregate early HBM
            # ~250-300GB/s/core), so loads are laid out across all three
            # rings in strict global deadline order:
            #   ACT:    ra x, rb slabs+x (issues finish before the first
            #           gelu queues on this engine), then all y stores
            #   sync:   ra slabs, then the sh bulk (x1, slabs, x2)
            #   GPSIMD: all down-weights (deadlines pushed past 55us by
            #           the phase interleave) + late sh x chunks
            # Keep >=4KB per partition-descriptor — smaller descriptors
            # are descriptor-rate-bound (~50GB/s/queue vs 300+ for 8KB+).
            def emit_loads(si):
                name, n, fp8, xd, wu, wd, yd = slots[si]
                dt = FP8 if fp8 else BF16
                wpool = wpool_r if fp8 else wpool_sh
                tt = SLOT_TT[name]
                wut = wpool.tile([128, DT, HT, 128], dt, tag="wup",
                                 name=f"wup_{name}")
                wdt = wpool.tile([128, DT, H], dt, tag="wdn",
                                 name=f"wdn_{name}")
                xts = []

                def load_slab(dj, halves=False, nd=1):
                    sw = HT * 128
                    hh = HT // 2
                    pieces = ([(0, hh), (hh, HT - hh)] if halves
                              else [(0, HT)])
                    for h0, hn in pieces:
                        nc.sync.dma_start(
                            out=wut[:, dj:dj + nd, h0:h0 + hn, :],
                            in_=wu[:, dj * sw + h0 * 128:
                                   (dj + nd - 1) * sw +
                                   (h0 + hn) * 128].rearrange(
                                "p (d h c) -> p d h c", d=nd, h=hn))

                def load_x(ci, eng, parts=2):
                    off, w = _chunks(n, tt)[ci]
                    xt = xpool.tile([128, HT, w], dt, tag="x",
                                    name=f"x_{name}_{off}")
                    xts.append(xt)
                    hh = HT // parts
                    for pi in range(parts):
                        h0 = pi * hh
                        eng.dma_start(
                            out=xt[:, h0:h0 + hh, :],
                            in_=xd[:, HT * off + h0 * w:
                                   HT * off + (h0 + hh) * w].rearrange(
                                "p (h w) -> p h w", h=hh))

                def load_wdn(gi, eng):
                    dg = DT // 2
                    g = gi * dg
                    eng.dma_start(
                        out=wdt[:, g:g + dg, :],
                        in_=wd[:, g * H:(g + dg) * H].rearrange(
                            "p (c h) -> p c h", c=dg))

                nchunks = len(_chunks(n, tt))
                if si == 0:
                    # first slot: slabs on sync, x on the GPSIMD ring —
                    # both rings ramp in parallel and the first matmul
                    # group only waits for slab0-half + one x quarter
                    load_slab(0, halves=True)
                    load_x(0, nc.gpsimd, parts=4)
                    # slabs 1-7 as 2-slab DMAs: 4KB descriptors clear the
                    # sync queue's descriptor-rate limit (~3-4x faster)
                    load_slab(1, nd=2)
                    load_slab(3, nd=2)
                    load_slab(5, nd=2)
                    load_slab(7)
                    for ci in range(1, nchunks):
                        load_x(ci, nc.gpsimd)
                    load_wdn(0, nc.gpsimd)
                    load_wdn(1, nc.gpsimd)
                elif name == "sh":
                    # bulk bf16 slot, both rings in deadline order.  sync:
                    # x1 before the slabs (all needed at slot start).
                    # GPSIMD: late x chunks BEFORE the down-weights — their
                    # pool-slot waits also throttle that ring through the
                    # early HBM crunch, and the ups-then-downs chunk order
                    # pushes the down-weight deadline to slot-start+56us.
                    load_x(0, nc.sync)
                    for dj in range(DT):
                        load_slab(dj)
                    if nchunks > 1:
                        load_x(1, nc.sync)
                    for ci in range(2, nchunks):
                        load_x(ci, nc.gpsimd)
                    load_wdn(0, nc.gpsimd)
                    load_wdn(1, nc.gpsimd)
                else:
                    for dj in range(DT):
                        load_slab(dj)
                    for ci in range(nchunks):
                        load_x(ci, nc.sync)
                    load_wdn(0, nc.gpsimd)
                    load_wdn(1, nc.gpsimd)
                return wut, wdt, xts

            # Dummy matmuls on scratch tiles: the PE HAM clock-gate only
            # lifts to 2.4 GHz after ~3.4us of sustained activity, so warm
            # it up while the first loads stream in.
            wlhs = xpool.tile([128, 128], BF16, tag="warm_l", bufs=1,
                              name="warm_lhs")
            wrhs = xpool.tile([128, TT], BF16, tag="warm_r", bufs=1,
                              name="warm_rhs")
            nc.vector.memset(wlhs[:], 0)
            nc.vector.memset(wrhs[:], 0)
            # preload the ACT gelu table (1.3us lazy load otherwise stalls
            # the first real activation)
            wgel = xpool.tile([128, 16], BF16, tag="warm_g", bufs=1,
                              name="warm_gelu")
            nc.scalar.activation(wgel[:], wrhs[:, :16], GELU)
            wps = ps1pool.tile([128, TT], F32, tag="ps1", name="warm_ps")
            for wi in range(WARM):
                nc.tensor.matmul(wps[:], lhsT=wlhs[:], rhs=wrhs[:],
                                 start=True, stop=True)

            loaded = [emit_loads(si) for si in range(len(slots))]

            def up_phase(slot, wut, xt, ci):
                name, n, fp8, xd, wu, wd, yd = slot
                off, w = _chunks(n, SLOT_TT[name])[ci]
                mid = mpool.tile([128, DT, w], FP8 if fp8 else BF16,
                                 tag="mid", name=f"mid_{name}_{off}")
                for dj in range(DT):
                    ps = ps1pool.tile([128, TT], F32, tag="ps1",
                                      name=f"ps1_{name}_{off}_{dj}")
                    if fp8:
                        for hq in range(HT // 2):
                            nc.tensor.matmul(
                                ps[:, :w],
                                lhsT=wut[:, dj, 2 * hq:2 * hq + 2, :],
                                rhs=xt[:, 2 * hq:2 * hq + 2, :],
                                start=(hq == 0),
                                stop=(hq == HT // 2 - 1),
                                perf_mode=DR,
                            )
                    else:
                        for hi in range(HT):
                            nc.tensor.matmul(
                                ps[:, :w],
                                lhsT=wut[:, dj, hi, :],
                                rhs=xt[:, hi, :],
                                start=(hi == 0),
                                stop=(hi == HT - 1),
                            )
                    nc.scalar.activation(
                        mid[:, dj, :], ps[:, :w], GELU,
                        scale=float(1 / (XS * WS)) if fp8 else 1.0)
                return mid

            def down_phase(slot, wdt, mid, ci, yg):
                name, n, fp8, xd, wu, wd, yd = slot
                off, w = _chunks(n, SLOT_TT[name])[ci]
                base = HT * off
                for hi in range(HT):
                    ps2 = ps2pool.tile([128, TT], F32, tag="ps2",
                                       name=f"ps2_{name}_{off}_{hi}")
                    if fp8:
                        for t in range(DT // 2):
                            nc.tensor.matmul(
                                ps2[:, :w],
                                lhsT=wdt[:, 2 * t:2 * t + 2,
                                         hi * 128:(hi + 1) * 128],
                                rhs=mid[:, 2 * t:2 * t + 2, :],
                                start=(t == 0),
                                stop=(t == DT // 2 - 1),
                                perf_mode=DR,
                            )
                    else:
                        for dj in range(DT):
                            nc.tensor.matmul(
                                ps2[:, :w],
                                lhsT=wdt[:, dj, hi * 128:(hi + 1) * 128],
                                rhs=mid[:, dj, :],
                                start=(dj == 0),
                                stop=(dj == DT - 1),
                            )
                    g = hi % yg
                    if g == 0:
                        yt = ypool.tile([128, yg, w],
                                        FP8 if fp8 else BF16, tag="y",
                                        name=f"y_{name}_{off}_{hi}")
                    if fp8:
                        nc.vector.tensor_scalar_mul(
                            yt[:, g, :], ps2[:, :w], float(1 / YS))
                    else:
                        nc.vector.tensor_copy(yt[:, g, :], ps2[:, :w])
                    if g == yg - 1:
                        # store on the ACT HWDGE ring (sync ring carries
                        # the loads)
                        lo = base + (hi - g) * w
                        nc.scalar.dma_start(
                            out=yd[:, lo:lo + yg * w].rearrange(
                                "p (h w) -> p h w", h=yg),
                            in_=yt[:])

            for si, slot in enumerate(slots):
                name, n, fp8, xd, wu, wd, yd = slot
                wut, wdt, xts = loaded[si]
                last_slot = si == len(slots) - 1
                nchunks = len(_chunks(n, SLOT_TT[name]))
                if name == "sh":
                    # all ups, then all downs: pushes the 4MB down-weight
                    # deadline from slot-start+14us to +56us, easing the
                    # early HBM crunch
                    mids = [up_phase(slot, wut, xts[ci], ci)
                            for ci in range(nchunks)]
                    for ci in range(nchunks):
                        down_phase(slot, wdt, mids[ci], ci, YG)
                else:
                    for ci in range(nchunks):
                        yg = 2 if (last_slot and ci == nchunks - 1) else YG
                        mid = up_phase(slot, wut, xts[ci], ci)
                        down_phase(slot, wdt, mid, ci, yg)
    nc.compile()
    return nc


def _pack_rows(a, nt):
    """[nt*128, m] row-major -> [128, nt*m] with per-partition contiguous
    (tile-major) layout."""
    m = a.shape[1]
    return np.ascontiguousarray(
        a.reshape(nt, 128, m).transpose(1, 0, 2).reshape(128, nt * m))


def _pack_slab(wu):
    """Up-weights [H, D] -> [128, DT*HT*128], dj-slab-major: element
    (k, dj, ht, c) = wu[ht*128+k, dj*128+c]."""
    a = wu.reshape(HT, 128, DT, 128).transpose(1, 2, 0, 3)
    return np.ascontiguousarray(a.reshape(128, DT * HT * 128))


def _pack_x(xTc, tt=TT):
    """[H, n] -> [128, HT*n] chunk-major."""
    n = xTc.shape[1]
    parts = [_pack_rows(xTc[:, off:off + w], HT)
             for off, w in _chunks(n, tt)]
    return np.ascontiguousarray(np.concatenate(parts, axis=1))


def _unpack_y(yflat, n, tt=TT):
    """[128, HT*n] chunk-major -> [n, H] (token-major)."""
    yflat = yflat.astype(np.float32)
    out = np.empty((n, H), np.float32)
    base = 0
    for off, w in _chunks(n, tt):
        blk = yflat[:, base:base + HT * w].reshape(128, HT, w)
        out[off:off + w] = blk.transpose(2, 1, 0).reshape(w, H)
        base += HT * w
    return out


def _to_fp8(a, scale):
    return np.clip(a * np.float32(scale), -240, 240).astype(NP_FP8)


def _route(x2d, w_router):
    """Top-2 routing, matching the reference's softmax-then-top_k."""
    logits = x2d @ w_router
    m = logits.max(-1, keepdims=True)
    e = np.exp(logits - m)
    probs = e / e.sum(-1, keepdims=True)
    rows = np.arange(x2d.shape[0])
    i1 = probs.argmax(-1)
    masked = probs.copy()
    masked[rows, i1] = -np.inf
    i2 = masked.argmax(-1)
    return probs, i1, i2


def kernel(x, Wsh_up, Wsh_down, Wrt_up, Wrt_down, W_router):
    global LAST_RESULTS
    x = np.asarray(x, np.float32)
    Wsh_up = np.asarray(Wsh_up, np.float32)
    Wsh_down = np.asarray(Wsh_down, np.float32)
    Wrt_up = np.asarray(Wrt_up, np.float32)
    Wrt_down = np.asarray(Wrt_down, np.float32)
    W_router = np.asarray(W_router, np.float32)

    x2d = x.reshape(T, H)
    probs, i1, i2 = _route(x2d, W_router)

    # token ids / gate values per routed expert
    ids, gates = [], []
    for e in range(E_RT):
        sel = np.where((i1 == e) | (i2 == e))[0]
        ids.append(sel)
        gates.append(probs[sel, e].astype(np.float32))

    # slot "rb" takes the 8 most-loaded experts, "ra" the 8 least-loaded,
    # so the two static capacities hug the actual counts ("ra" usually
    # becomes a single full-width 512 chunk and runs first).
    order = sorted(range(E_RT), key=lambda e: -len(ids[e]))
    slot_experts = {"rb": order[:CORES], "ra": order[CORES:]}
    caps = {}
    for slot in ("ra", "rb"):
        mx = max(len(ids[e]) for e in slot_experts[slot])
        caps[slot] = max(512, -(-mx // 32) * 32)

    key = (caps["ra"], caps["rb"])
    if key not in _prog_cache:
        _prog_cache[key] = _build_program(*key)
    nc = _prog_cache[key]

    xbf = x2d.astype(NP_BF16)

    in_maps = []
    for c in range(CORES):
        se, q = c % E_SH, c // E_SH
        m = {
            "x_sh": _pack_x(np.ascontiguousarray(
                xbf[q * N_SH:(q + 1) * N_SH].T), SLOT_TT["sh"]),
            "wup_sh": _pack_slab(Wsh_up[se]).astype(NP_BF16),
            "wdn_sh": _pack_rows(Wsh_down[se], DT).astype(NP_BF16),
        }
        for slot in ("ra", "rb"):
            e = slot_experts[slot][c]
            sel = ids[e]
            cap = caps[slot]
            xe = np.zeros((H, cap), NP_FP8)
            xe[:, :len(sel)] = _to_fp8(x2d[sel].T, XS)
            m[f"x_{slot}"] = _pack_x(xe, SLOT_TT[slot])
            m[f"wup_{slot}"] = _to_fp8(_pack_slab(Wrt_up[e]), WS)
            m[f"wdn_{slot}"] = _to_fp8(_pack_rows(Wrt_down[e], DT), WS)
        in_maps.append(m)

    res = run_bass_kernel_spmd(nc, in_maps, core_ids=list(range(CORES)))
    LAST_RESULTS = res

    out = np.zeros((T, H), np.float32)
    for c in range(CORES):
        q = c // E_SH
        out[q * N_SH:(q + 1) * N_SH] += _unpack_y(res.results[c]["y_sh"],
                                                  N_SH, SLOT_TT["sh"])
    for slot in ("ra", "rb"):
        for c in range(CORES):
            e = slot_experts[slot][c]
            sel = ids[e]
            y = _unpack_y(res.results[c][f"y_{slot}"], caps[slot],
                          SLOT_TT[slot])
            out[sel] += (gates[e] * (YS / WS))[:, None] * y[:len(sel)]
    return out.reshape(B, S, H)
